# revision 17
# baseline (speedup 1.0000x reference)
"""GraphSAGE 3-layer GNN forward pass on 8 Trainium2 NeuronCores.

Sharding: nodes split by range across 8 cores (graph/data parallel).
Per layer the message table z = h @ Wl is computed shard-wise (bf16, rows
padded to 128 cols = 256B) and AllGathered into a replicated DRAM table;
each core aggregates the edges whose dst is in its shard: dma_gather pulls
z[src] rows (256B) into SBUF and a one-hot matmul on the tensor engine does
the segment-sum into PSUM (feature-major for layers 1/2, node-major for
layer 3). Mean-normalization (1/deg), the self term h @ Wr (fp32), BatchNorm
(stats AllReduced), ReLU and a batched log_softmax run on vector/scalar.
Hidden activations h1/h2 and the layer-3 self term stay SBUF-resident.

int16 gather indices only reach 32768 rows, so the 100352-row table is
processed in 4 buckets of 25088 rows (slices of one AllGathered tensor).

Edge schedule: per (group of GSIZE dst tiles, src bucket) call, edges are
sorted by dst tile and packed contiguously (chunks may straddle tiles; the
union-of-cores (chunk, tile) piece schedule drives per-piece one-hots whose
dstrel is -1 outside the piece). Trailing idx slots are 0 (gathered then
zeroed by the one-hot); contiguous packing keeps the pad at ~4%.
Gather calls rotate over 4 SWDGE queues so the 4 Q7 core pairs overlap
descriptor generation (the per-edge Q7 descriptor work is the wall); small
GSIZE keeps calls short enough for the engine's in-flight window.
The z phases batch 4 tiles per PSUM bank and one PSUM->SBUF copy per quad.
"""

import numpy as np

# ---------------- problem constants (hardcoded per contract) ----------------
N = 100000
E = 1600000
FIN = 200
NCORES = 8
NPC = N // NCORES            # 12500 nodes per core
NT = 98                      # dst tiles of 128 nodes per core
NPAD = NT * 128              # 12544
SHARD = NPAD                 # table rows contributed per core
TROWS = SHARD * NCORES       # 100352
NBUCK = 4
BROWS = TROWS // NBUCK       # 25088 (< 32768, int16-safe)
F1, F2, F3 = 64, 32, 17
EPS = 1e-5

# ---------------- tunables ----------------
NQ = 4                # SWDGE queues used for gather calls
NSQ = 4               # queues declared
GSIZE = 4             # dst tiles per PSUM accumulation group
PBATCH = 16           # pieces per one-hot build DVE op
GBUFS = 8             # gather buffer pool depth


def _bf16(x):
    import ml_dtypes
    return np.asarray(x).astype(ml_dtypes.bfloat16)


def _wrap16(idx_flat):
    """dma_gather index layout: position i -> partition i%16, col i//16,
    replicated across the 8 q7 core pairs (128 partitions)."""
    n = idx_flat.shape[0]
    w = idx_flat.reshape(n // 16, 16).T.copy()
    return np.tile(w, (8, 1))


def _preprocess(edge_index):
    src = np.asarray(edge_index[0], dtype=np.int64)
    dst = np.asarray(edge_index[1], dtype=np.int64)
    trow = (src // NPC) * SHARD + (src % NPC)   # global table row of src
    bucket = trow // BROWS
    rel = trow - bucket * BROWS

    dst_core = dst // NPC
    dloc = dst - dst_core * NPC
    tile_e = dloc >> 7
    dstrel_e = dloc & 127

    groups = [list(range(g, min(g + GSIZE, NT))) for g in range(0, NT, GSIZE)]
    ngroups = len(groups)
    gi_of_tile = np.zeros(NT, np.int64)
    for gi, g in enumerate(groups):
        for t in g:
            gi_of_tile[t] = gi
    ncalls = ngroups * NBUCK

    per_core = []
    cnt = np.zeros((NCORES, ncalls), np.int64)
    for c in range(NCORES):
        m = dst_core == c
        gi_e = gi_of_tile[tile_e[m]]
        call_e = gi_e * NBUCK + bucket[m]
        order = np.lexsort((tile_e[m], call_e))
        per_core.append({
            "call": call_e[order],
            "tile": tile_e[m][order],
            "rel": rel[m][order],
            "dstrel": dstrel_e[m][order],
            "cnt_node": np.bincount(dloc[m], minlength=NPC),
        })
        cnt[c] = np.bincount(call_e, minlength=ncalls)

    nch_call = np.maximum((cnt.max(axis=0) + 127) // 128, 1)   # [ncalls]

    # union piece schedule: per call, sorted set of (chunk, tile)
    pieces_per_call = []
    for ci in range(ncalls):
        pieces = set()
        for c in range(NCORES):
            ck = per_core[c]
            sel = np.nonzero(ck["call"] == ci)[0]
            if len(sel) == 0:
                continue
            pos = np.arange(len(sel))
            chunks = pos >> 7
            tiles = ck["tile"][sel]
            pieces.update(zip(chunks.tolist(), tiles.tolist()))
        gi = ci // NBUCK
        if not pieces:
            pieces = {(0, groups[gi][0])}
        pieces_per_call.append(sorted(pieces))

    piece_chunk, piece_tile = [], []
    ps_fm, pp_fm, ps_nm, pp_nm = [], [], [], []
    call_pstart = np.zeros(ncalls + 1, np.int64)
    for gi in range(ngroups):
        g = groups[gi]
        gp = []
        for b in range(NBUCK):
            ci = gi * NBUCK + b
            call_pstart[ci] = len(piece_chunk)
            for (ch, t) in pieces_per_call[ci]:
                gp.append((len(piece_chunk), t))
                piece_chunk.append(ch)
                piece_tile.append(t)
        first_bk, last_bk = {}, {}
        for idx, t in gp:
            bk = (t - g[0]) // 4
            first_bk.setdefault(bk, idx)
            last_bk[bk] = idx
        for idx, t in gp:
            bk = (t - g[0]) // 4
            ps_fm.append(first_bk[bk] == idx)
            pp_fm.append(last_bk[bk] == idx)
            ps_nm.append(idx == gp[0][0])
            pp_nm.append(idx == gp[-1][0])
    call_pstart[ncalls] = len(piece_chunk)
    npieces = len(piece_chunk)

    idx_cols = int(nch_call.sum()) * 8
    idx_all = np.zeros((NCORES, 128, idx_cols), np.int16)
    dstrel_all = np.full((NCORES, 128, npieces), -1.0, np.float32)
    rcnt_row = np.zeros((NCORES, NPAD), np.float32)
    rcnt_nm = np.zeros((NCORES, 128, NT), np.float32)
    for c in range(NCORES):
        ck = per_core[c]
        idx_parts = []
        for ci in range(ncalls):
            sel = np.nonzero(ck["call"] == ci)[0]
            nidx = int(nch_call[ci]) * 128
            flat = np.zeros(nidx, np.int16)
            flat[:len(sel)] = ck["rel"][sel].astype(np.int16)
            idx_parts.append(_wrap16(flat))
            pos = np.arange(len(sel))
            chunks = pos >> 7
            lanes = pos & 127
            tiles = ck["tile"][sel]
            p0 = call_pstart[ci]
            pmap = {(ch, t): k for k, (ch, t) in enumerate(pieces_per_call[ci])}
            pidx = np.fromiter((pmap[(ch, t)] for ch, t in zip(chunks.tolist(), tiles.tolist())),
                               np.int64, count=len(sel))
            dstrel_all[c][lanes, p0 + pidx] = ck["dstrel"][sel].astype(np.float32)
        idx_all[c] = np.concatenate(idx_parts, axis=1)
        rc_pad = np.ones(NPAD, np.float32)
        rc_pad[:NPC] = 1.0 / np.maximum(ck["cnt_node"], 1).astype(np.float32)
        rcnt_row[c] = rc_pad
        rcnt_nm[c] = rc_pad.reshape(NT, 128).T

    return {
        "groups": groups, "ncalls": ncalls, "nch_call": nch_call,
        "call_pstart": call_pstart, "npieces": npieces,
        "piece_chunk": np.array(piece_chunk), "piece_tile": np.array(piece_tile),
        "ps_fm": np.array(ps_fm), "pp_fm": np.array(pp_fm),
        "ps_nm": np.array(ps_nm), "pp_nm": np.array(pp_nm),
        "idx_all": idx_all, "idx_cols": idx_cols, "dstrel_all": dstrel_all,
        "rcnt_row": rcnt_row, "rcnt_nm": rcnt_nm,
    }


def _build_program(pp):
    import concourse.bacc as bacc
    import concourse.tile as tile
    import concourse.bass as bass
    import concourse.mybir as mybir

    f32 = mybir.dt.float32
    bf16 = mybir.dt.bfloat16
    AX = mybir.AxisListType
    ALU = mybir.AluOpType
    ACT = mybir.ActivationFunctionType

    groups = pp["groups"]
    ngroups = len(groups)
    ncalls = pp["ncalls"]
    nch_call = pp["nch_call"]
    call_pstart = pp["call_pstart"]
    npieces = pp["npieces"]
    piece_chunk = pp["piece_chunk"]
    piece_tile = pp["piece_tile"]
    ps_fm, pp_fm = pp["ps_fm"], pp["pp_fm"]
    ps_nm, pp_nm = pp["ps_nm"], pp["pp_nm"]
    idx_cols = pp["idx_cols"]
    MCC = int(nch_call.max())
    GW = GSIZE * 128

    nc = bacc.Bacc("TRN2", target_bir_lowering=False, debug=False,
                   num_devices=NCORES, num_swdge_queues=NSQ)

    # ---------------- I/O ----------------
    t_xT = nc.dram_tensor("xT", [FIN, NPAD], bf16, kind="ExternalInput")
    t_idx = nc.dram_tensor("gidx", [128, idx_cols], mybir.dt.int16, kind="ExternalInput")
    t_dstrel = nc.dram_tensor("dstrel", [128, npieces], bf16, kind="ExternalInput")
    t_rcnt_nm = nc.dram_tensor("rcnt_nm", [128, NT], f32, kind="ExternalInput")
    t_rcnt_fm = nc.dram_tensor("rcnt_fm", [64, NPAD], f32, kind="ExternalInput")
    t_iota = nc.dram_tensor("iota", [128, 128], bf16, kind="ExternalInput")
    t_W1l = nc.dram_tensor("W1l", [FIN, F1], bf16, kind="ExternalInput")
    t_W1r = nc.dram_tensor("W1r", [FIN, F1], bf16, kind="ExternalInput")
    t_W2l = nc.dram_tensor("W2l", [F1, F2], bf16, kind="ExternalInput")
    t_W2r = nc.dram_tensor("W2r", [F1, F2], bf16, kind="ExternalInput")
    t_W3l = nc.dram_tensor("W3l", [F2, F3], bf16, kind="ExternalInput")
    t_W3r = nc.dram_tensor("W3r", [F2, F3], bf16, kind="ExternalInput")
    t_g1 = nc.dram_tensor("g1", [F1, 1], f32, kind="ExternalInput")
    t_be1 = nc.dram_tensor("be1", [F1, 1], f32, kind="ExternalInput")
    t_g2 = nc.dram_tensor("g2", [F2, 1], f32, kind="ExternalInput")
    t_be2 = nc.dram_tensor("be2", [F2, 1], f32, kind="ExternalInput")
    t_b3 = nc.dram_tensor("b3rep", [128, F3], f32, kind="ExternalInput")
    t_out = nc.dram_tensor("out", [NPAD, F3], f32, kind="ExternalOutput")

    shard = [nc.dram_tensor(f"shard{li}", [SHARD, 128], bf16, kind="Internal")
             for li in range(3)]
    zfull = [nc.dram_tensor(f"zfull{li}", [TROWS, 128], bf16,
                            kind="Internal", addr_space="Shared")
             for li in range(3)]
    zrT1_d = nc.dram_tensor("zrT1", [64, NPAD], f32, kind="Internal")
    zrT2_d = nc.dram_tensor("zrT2", [F2, NPAD], f32, kind="Internal")
    bn_in1 = nc.dram_tensor("bn_in1", [F1, 2], f32, kind="Internal")
    bn_out1 = nc.dram_tensor("bn_out1", [F1, 2], f32, kind="Internal", addr_space="Shared")
    bn_in2 = nc.dram_tensor("bn_in2", [F2, 2], f32, kind="Internal")
    bn_out2 = nc.dram_tensor("bn_out2", [F2, 2], f32, kind="Internal", addr_space="Shared")

    RG = [list(range(NCORES))]

    with tile.TileContext(nc) as tc:
        with tc.tile_pool(name="const", bufs=1) as constp, \
             tc.tile_pool(name="wpool", bufs=1) as wpool, \
             tc.tile_pool(name="stage", bufs=2) as stagep, \
             tc.tile_pool(name="sm3", bufs=3) as sm3p, \
             tc.tile_pool(name="slab", bufs=2) as slabp, \
             tc.tile_pool(name="gbuf", bufs=GBUFS) as gbufp, \
             tc.tile_pool(name="pbuf", bufs=3) as pbufp, \
             tc.tile_pool(name="zpsum", bufs=2, space="PSUM") as zpsum, \
             tc.tile_pool(name="spsum", bufs=2, space="PSUM") as spsum, \
             tc.tile_pool(name="small", bufs=1) as smallp:

            # ---- constants
            iota = constp.tile([128, 128], bf16)
            nc.sync.dma_start(iota[:], t_iota.ap())
            idx_t = constp.tile([128, idx_cols], mybir.dt.int16)
            nc.sync.dma_start(idx_t[:], t_idx.ap())
            dstrel_t = constp.tile([128, npieces], bf16)
            nc.sync.dma_start(dstrel_t[:], t_dstrel.ap())
            rcnt_nm_t = constp.tile([128, NT], f32)
            nc.sync.dma_start(rcnt_nm_t[:], t_rcnt_nm.ap())
            b3rep = constp.tile([128, F3], f32)
            nc.sync.dma_start(b3rep[:], t_b3.ap())

            def wload(name, tt, shape, dt=bf16):
                w = wpool.tile(shape, dt, tag=name)
                nc.sync.dma_start(w[:], tt)
                return w

            W1l_a = wload("w1la", t_W1l.ap()[:128], [128, F1])
            W1l_b = wload("w1lb", t_W1l.ap()[128:], [72, F1])
            W1r_a = wload("w1ra", t_W1r.ap()[:128], [128, F1])
            W1r_b = wload("w1rb", t_W1r.ap()[128:], [72, F1])
            W2l_t = wload("w2l", t_W2l.ap(), [F1, F2])
            W2r_t = wload("w2r", t_W2r.ap(), [F1, F2])
            W3l_t = wload("w3l", t_W3l.ap(), [F2, F3])
            W3r_t = wload("w3r", t_W3r.ap(), [F2, F3])
            g1_t = wload("g1", t_g1.ap(), [F1, 1], f32)
            be1_t = wload("be1", t_be1.ap(), [F1, 1], f32)
            g2_t = wload("g2", t_g2.ap(), [F2, 1], f32)
            be2_t = wload("be2", t_be2.ap(), [F2, 1], f32)

            # SBUF-resident hidden activations and layer-3 self term
            h1sb = smallp.tile([F1, NPAD], bf16, tag="h1sb")
            h2sb = smallp.tile([F2, NPAD], bf16, tag="h2sb")
            zr3sb = smallp.tile([128, NT * F3], f32, tag="zr3sb")

            # pre-zero rotating pools whose stale regions reach matmuls/DMA
            for _ in range(GBUFS):
                gz = gbufp.tile([128, MCC, 128], bf16, tag="gb")
                nc.vector.memzero(gz[:])
            for _ in range(3):
                zz = sm3p.tile([128, GSIZE, 128], bf16, tag="zslab")
                nc.vector.memzero(zz[:])

            # ================= layer-1 z phase =================
            with nc.named_scope("L1z"):
                for gi, g in enumerate(groups):
                    gw = len(g) * 128
                    c0 = g[0] * 128
                    xa = slabp.tile([128, GW], bf16, tag="xa")
                    xb = slabp.tile([72, GW], bf16, tag="xb")
                    nc.sync.dma_start(xa[:, :gw], t_xT.ap()[:128, c0:c0 + gw])
                    nc.sync.dma_start(xb[:, :gw], t_xT.ap()[128:, c0:c0 + gw])
                    zr_sl = stagep.tile([64, GW], f32, tag="zrslab")
                    zsl = sm3p.tile([128, GSIZE, 128], bf16, tag="zslab")
                    for q in range(0, len(g), 4):
                        qt = min(4, len(g) - q)
                        pz4 = zpsum.tile([128, 512], f32, tag="zps")
                        pr4 = zpsum.tile([128, 512], f32, tag="zps")
                        for k in range(qt):
                            ti = q + k
                            xs_a = xa[:, ti * 128:(ti + 1) * 128]
                            xs_b = xb[:, ti * 128:(ti + 1) * 128]
                            nc.tensor.matmul(pz4[:, k * 64:k * 64 + F1], xs_a, W1l_a[:],
                                             start=(k == 0), stop=False,
                                             skip_group_check=True)
                            nc.tensor.matmul(pz4[:, k * 64:k * 64 + F1], xs_b, W1l_b[:],
                                             start=False, stop=(k == qt - 1),
                                             skip_group_check=True)
                            nc.tensor.matmul(pr4[:F1, k * 128:(k + 1) * 128], W1r_a[:], xs_a,
                                             start=(k == 0), stop=False,
                                             skip_group_check=True)
                            nc.tensor.matmul(pr4[:F1, k * 128:(k + 1) * 128], W1r_b[:], xs_b,
                                             start=False, stop=(k == qt - 1),
                                             skip_group_check=True)
                        nc.scalar.copy(
                            zsl[:, q:q + qt, 0:F1],
                            pz4[:, :qt * 64].rearrange("p (t f) -> p t f", f=64))
                        nc.scalar.copy(zr_sl[:, q * 128:(q + qt) * 128],
                                       pr4[:F1, :qt * 128])
                    nc.sync.dma_start(
                        shard[0].ap()[c0:c0 + gw].rearrange("(t p) f -> p t f", p=128),
                        zsl[:, :len(g), :])
                    nc.sync.dma_start(zrT1_d.ap()[:, c0:c0 + gw], zr_sl[:, :gw])

            def allgather(li, scope):
                with nc.named_scope(scope):
                    nc.gpsimd.collective_compute(
                        "AllGather", ALU.bypass, replica_groups=RG,
                        ins=[shard[li].ap()], outs=[zfull[li].ap()])

            allgather(0, "AG1")

            # ========== generic gather/aggregate ==========
            def agg_layer(li, Fw, fm, zr_src, h_sink, scope, final_cb=None):
                stat_parts = []
                pstart = ps_fm if fm else ps_nm
                pstop = pp_fm if fm else pp_nm
                with nc.named_scope(scope):
                    cur_ps = None
                    for ci in range(ncalls):
                        gi, b = ci // NBUCK, ci % NBUCK
                        g = groups[gi]
                        gw = len(g) * 128
                        c0 = g[0] * 128
                        nch = int(nch_call[ci])
                        qs8 = int(nch_call[:ci].sum()) * 8
                        if b == 0:
                            if fm:
                                cur_ps = spsum.tile([Fw, GW], f32, tag="sacc")
                            else:
                                cur_ps = spsum.tile([128, GSIZE * F3], f32, tag="sacc3")
                        ps = cur_ps
                        gb = gbufp.tile([128, MCC, 128], bf16, tag="gb")
                        nc.gpsimd.dma_gather(
                            out_ap=gb[:, :nch, :],
                            in_ap=zfull[li].ap()[b * BROWS:(b + 1) * BROWS],
                            idxs_ap=idx_t[:, qs8:qs8 + nch * 8],
                            num_idxs=nch * 128, num_idxs_reg=nch * 128,
                            elem_size=128, single_packet=False,
                            queue_num=ci % NQ)
                        p0 = int(call_pstart[ci])
                        pend = int(call_pstart[ci + 1])
                        poff = p0
                        while poff < pend:
                            bs = min(PBATCH, pend - poff)
                            P = pbufp.tile([128, PBATCH * 128], bf16, tag="P")
                            nc.vector.tensor_tensor(
                                out=P[:, :bs * 128].rearrange("p (g v) -> p g v", g=bs),
                                in0=dstrel_t[:, poff:poff + bs]
                                    .to_broadcast([128, bs, 128]),
                                in1=iota[:].rearrange("p (o v) -> p o v", o=1)
                                    .to_broadcast([128, bs, 128]),
                                op=ALU.is_equal)
                            for j in range(bs):
                                pj = poff + j
                                ch = int(piece_chunk[pj])
                                t = int(piece_tile[pj])
                                ti = t - g[0]
                                first = bool(pstart[pj])
                                last = bool(pstop[pj])
                                if fm:
                                    nc.tensor.matmul(
                                        ps[:, ti * 128:(ti + 1) * 128],
                                        gb[:, ch, 0:Fw],
                                        P[:, j * 128:(j + 1) * 128],
                                        start=first, stop=last, skip_group_check=True)
                                else:
                                    nc.tensor.matmul(
                                        ps[:, ti * F3:(ti + 1) * F3],
                                        P[:, j * 128:(j + 1) * 128],
                                        gb[:, ch, 0:F3],
                                        start=first, stop=last, skip_group_check=True)
                            poff += bs
                        if b == NBUCK - 1:
                            if fm:
                                rc_sl = slabp.tile([64, GW], f32, tag="rcsl")
                                nc.sync.dma_start(rc_sl[:Fw, :gw], t_rcnt_fm.ap()[:Fw, c0:c0 + gw])
                                zr_sl2 = slabp.tile([64, GW], f32, tag="zrsl2")
                                nc.sync.dma_start(zr_sl2[:Fw, :gw], zr_src[:, c0:c0 + gw])
                                hsl = stagep.tile([64, GW], f32, tag="hsl")
                                nc.vector.tensor_mul(hsl[:Fw, :gw], ps[:, :gw], rc_sl[:Fw, :gw])
                                nc.vector.tensor_add(h_sink[:, c0:c0 + gw],
                                                     hsl[:Fw, :gw], zr_sl2[:Fw, :gw])
                                s_p = smallp.tile([Fw, 2], f32, tag=f"stat_{scope}_{gi}")
                                nc.vector.tensor_reduce(s_p[:, 0:1], h_sink[:, c0:c0 + gw],
                                                        axis=AX.X, op=ALU.add)
                                sq_scr = stagep.tile([64, GW], bf16, tag="sqscr")
                                nc.scalar.activation(sq_scr[:Fw, :gw], h_sink[:, c0:c0 + gw],
                                                     ACT.Square, accum_out=s_p[:, 1:2])
                                stat_parts.append(s_p)
                            else:
                                final_cb(ps, g, gw, c0)
                return stat_parts

            def bn_finalize(stat_parts, Fw, bn_in, bn_out, g_t, be_t, scope):
                with nc.named_scope(scope):
                    np_ = len(stat_parts)
                    stk = smallp.tile([Fw, 2 * np_], f32, tag=f"stk_{scope}")
                    for i, s_p in enumerate(stat_parts):
                        nc.vector.tensor_copy(stk[:, 2 * i:2 * i + 2], s_p[:])
                    tot = smallp.tile([Fw, 2], f32, tag=f"tot_{scope}")
                    v = stk[:].rearrange("f (i two) -> f two i", two=2)
                    nc.vector.tensor_reduce(tot[:, 0:1], v[:, 0:1, :], axis=AX.X, op=ALU.add)
                    nc.vector.tensor_reduce(tot[:, 1:2], v[:, 1:2, :], axis=AX.X, op=ALU.add)
                    nc.sync.dma_start(bn_in.ap(), tot[:])
                    nc.gpsimd.collective_compute(
                        "AllReduce", ALU.add, replica_groups=RG,
                        ins=[bn_in.ap()], outs=[bn_out.ap()])
                    red = smallp.tile([Fw, 2], f32, tag=f"red_{scope}")
                    nc.sync.dma_start(red[:], bn_out.ap())
                    mean = smallp.tile([Fw, 1], f32, tag=f"mean_{scope}")
                    nc.vector.tensor_scalar_mul(mean[:], red[:, 0:1], 1.0 / N)
                    ex2 = smallp.tile([Fw, 1], f32, tag=f"ex2_{scope}")
                    nc.vector.tensor_scalar_mul(ex2[:], red[:, 1:2], 1.0 / N)
                    var = smallp.tile([Fw, 1], f32, tag=f"var_{scope}")
                    nc.vector.tensor_mul(var[:], mean[:], mean[:])
                    nc.vector.tensor_sub(var[:], ex2[:], var[:])
                    nc.vector.tensor_scalar_add(var[:], var[:], EPS)
                    std = smallp.tile([Fw, 1], f32, tag=f"std_{scope}")
                    nc.scalar.sqrt(std[:], var[:])
                    rstd = smallp.tile([Fw, 1], f32, tag=f"rstd_{scope}")
                    nc.vector.reciprocal(rstd[:], std[:])
                    scal = smallp.tile([Fw, 1], f32, tag=f"scal_{scope}")
                    nc.vector.tensor_mul(scal[:], g_t[:], rstd[:])
                    shift = smallp.tile([Fw, 1], f32, tag=f"shift_{scope}")
                    nc.vector.tensor_mul(shift[:], mean[:], scal[:])
                    nc.vector.tensor_sub(shift[:], be_t[:], shift[:])
                    return scal, shift

            stats1 = agg_layer(0, F1, True, zrT1_d.ap(), h1sb, "L1agg")
            scal1, shift1 = bn_finalize(stats1, F1, bn_in1, bn_out1, g1_t, be1_t, "BN1")

            # ================= layer-2 z phase =================
            with nc.named_scope("L2z"):
                for gi, g in enumerate(groups):
                    gw = len(g) * 128
                    c0 = g[0] * 128
                    hsb = slabp.tile([64, GW], bf16, tag="hsb")
                    nc.scalar.activation(hsb[:F1, :gw], h1sb[:, c0:c0 + gw], ACT.Relu,
                                         bias=shift1[:], scale=scal1[:])
                    if g[-1] == NT - 1:
                        nc.vector.memzero(hsb[:F1, NPC - c0:gw])
                    zr_sl = stagep.tile([64, GW], f32, tag="zrslab")
                    zsl = sm3p.tile([128, GSIZE, 128], bf16, tag="zslab")
                    for q in range(0, len(g), 4):
                        qt = min(4, len(g) - q)
                        pz4 = zpsum.tile([128, 512], f32, tag="zps")
                        pr4 = zpsum.tile([128, 512], f32, tag="zps")
                        for k in range(qt):
                            ti = q + k
                            hst = hsb[:F1, ti * 128:(ti + 1) * 128]
                            nc.tensor.matmul(pz4[:, k * 64:k * 64 + F2], hst, W2l_t[:],
                                             start=(k == 0), stop=(k == qt - 1),
                                             skip_group_check=True)
                            nc.tensor.matmul(pr4[:F2, k * 128:(k + 1) * 128], W2r_t[:], hst,
                                             start=(k == 0), stop=(k == qt - 1),
                                             skip_group_check=True)
                        nc.scalar.copy(
                            zsl[:, q:q + qt, 0:F2],
                            pz4[:, :qt * 64].rearrange("p (t f) -> p t f", f=64)[:, :, 0:F2])
                        nc.scalar.copy(zr_sl[:F2, q * 128:(q + qt) * 128],
                                       pr4[:F2, :qt * 128])
                    nc.sync.dma_start(
                        shard[1].ap()[c0:c0 + gw].rearrange("(t p) f -> p t f", p=128),
                        zsl[:, :len(g), :])
                    nc.sync.dma_start(zrT2_d.ap()[:, c0:c0 + gw], zr_sl[:F2, :gw])

            allgather(1, "AG2")

            stats2 = agg_layer(1, F2, True, zrT2_d.ap(), h2sb, "L2agg")
            scal2, shift2 = bn_finalize(stats2, F2, bn_in2, bn_out2, g2_t, be2_t, "BN2")

            # ================= layer-3 z phase =================
            with nc.named_scope("L3z"):
                for gi, g in enumerate(groups):
                    gw = len(g) * 128
                    c0 = g[0] * 128
                    hsb = slabp.tile([64, GW], bf16, tag="hsb")
                    nc.scalar.activation(hsb[:F2, :gw], h2sb[:, c0:c0 + gw], ACT.Relu,
                                         bias=shift2[:], scale=scal2[:])
                    if g[-1] == NT - 1:
                        nc.vector.memzero(hsb[:F2, NPC - c0:gw])
                    zsl = sm3p.tile([128, GSIZE, 128], bf16, tag="zslab")
                    for q in range(0, len(g), 4):
                        qt = min(4, len(g) - q)
                        pz4 = zpsum.tile([128, 512], f32, tag="zps")
                        pr4 = zpsum.tile([128, 512], f32, tag="zps")
                        for k in range(qt):
                            ti = q + k
                            hst = hsb[:F2, ti * 128:(ti + 1) * 128]
                            nc.tensor.matmul(pz4[:, k * 64:k * 64 + F3], hst, W3l_t[:],
                                             start=(k == 0), stop=(k == qt - 1),
                                             skip_group_check=True)
                            nc.tensor.matmul(pr4[:, k * 64:k * 64 + F3], hst, W3r_t[:],
                                             start=(k == 0), stop=(k == qt - 1),
                                             skip_group_check=True)
                        nc.scalar.copy(
                            zsl[:, q:q + qt, 0:F3],
                            pz4[:, :qt * 64].rearrange("p (t f) -> p t f", f=64)[:, :, 0:F3])
                        nc.scalar.copy(
                            zr3sb[:, (g[0] + q) * F3:(g[0] + q + qt) * F3]
                                .rearrange("p (t f) -> p t f", f=F3),
                            pr4[:, :qt * 64].rearrange("p (t f) -> p t f", f=64)[:, :, 0:F3])
                    nc.sync.dma_start(
                        shard[2].ap()[c0:c0 + gw].rearrange("(t p) f -> p t f", p=128),
                        zsl[:, :len(g), :])

            allgather(2, "AG3")

            def l3_final(ps, g, gw, c0):
                ng = len(g)
                W = ng * F3
                zr3v = zr3sb[:, g[0] * F3:g[0] * F3 + W]
                h3 = sm3p.tile([128, GSIZE * F3], f32, tag="h3")
                nc.vector.tensor_tensor(
                    out=h3[:, :W].rearrange("p (t f) -> p t f", f=F3),
                    in0=ps[:, :W].rearrange("p (t f) -> p t f", f=F3),
                    in1=rcnt_nm_t[:, g[0]:g[0] + ng]
                        .rearrange("p (t o) -> p t o", o=1)
                        .to_broadcast([128, ng, F3]),
                    op=ALU.mult)
                nc.vector.tensor_add(h3[:, :W], h3[:, :W], zr3v)
                nc.vector.tensor_tensor(
                    out=h3[:, :W].rearrange("p (t f) -> p t f", f=F3),
                    in0=h3[:, :W].rearrange("p (t f) -> p t f", f=F3),
                    in1=b3rep[:, :F3].rearrange("p (o f) -> p o f", o=1)
                        .to_broadcast([128, ng, F3]),
                    op=ALU.add)
                mx = sm3p.tile([128, GSIZE], f32, tag="mx")
                nc.vector.tensor_reduce(
                    mx[:, :ng], h3[:, :W].rearrange("p (t f) -> p t f", f=F3),
                    axis=AX.X, op=ALU.max)
                nc.vector.tensor_tensor(
                    out=h3[:, :W].rearrange("p (t f) -> p t f", f=F3),
                    in0=h3[:, :W].rearrange("p (t f) -> p t f", f=F3),
                    in1=mx[:, :ng].rearrange("p (t o) -> p t o", o=1)
                        .to_broadcast([128, ng, F3]),
                    op=ALU.subtract)
                ex = sm3p.tile([128, GSIZE * F3], f32, tag="ex")
                nc.scalar.activation(ex[:, :W], h3[:, :W], ACT.Exp)
                se = sm3p.tile([128, GSIZE], f32, tag="se")
                nc.vector.tensor_reduce(
                    se[:, :ng], ex[:, :W].rearrange("p (t f) -> p t f", f=F3),
                    axis=AX.X, op=ALU.add)
                ls = sm3p.tile([128, GSIZE], f32, tag="ls")
                nc.scalar.activation(ls[:, :ng], se[:, :ng], ACT.Ln)
                nc.vector.tensor_tensor(
                    out=h3[:, :W].rearrange("p (t f) -> p t f", f=F3),
                    in0=h3[:, :W].rearrange("p (t f) -> p t f", f=F3),
                    in1=ls[:, :ng].rearrange("p (t o) -> p t o", o=1)
                        .to_broadcast([128, ng, F3]),
                    op=ALU.subtract)
                nc.sync.dma_start(
                    t_out.ap()[c0:c0 + gw].rearrange("(t p) f -> p t f", p=128),
                    h3[:, :W].rearrange("p (t f) -> p t f", f=F3))

            agg_layer(2, F3, False, None, None, "L3agg", final_cb=l3_final)

    nc.compile()
    return nc


_PROG_CACHE = {}


def _in_maps(pp, inputs):
    x = np.asarray(inputs["x"], np.float32)
    iota = np.broadcast_to(np.arange(128, dtype=np.float32)[None, :], (128, 128))
    b3rep = np.broadcast_to(np.asarray(inputs["b3"], np.float32)[None, :], (128, F3)).copy()
    common = {
        "iota": _bf16(iota),
        "W1l": _bf16(inputs["W1l"]),
        "W1r": _bf16(inputs["W1r"]),
        "W2l": _bf16(inputs["W2l"]),
        "W2r": _bf16(inputs["W2r"]),
        "W3l": _bf16(inputs["W3l"]),
        "W3r": _bf16(inputs["W3r"]),
        "g1": np.asarray(inputs["g1"], np.float32)[:, None].copy(),
        "be1": np.asarray(inputs["be1"], np.float32)[:, None].copy(),
        "g2": np.asarray(inputs["g2"], np.float32)[:, None].copy(),
        "be2": np.asarray(inputs["be2"], np.float32)[:, None].copy(),
        "b3rep": b3rep,
    }
    in_maps = []
    for c in range(NCORES):
        xT = np.zeros((FIN, NPAD), np.float32)
        xT[:, :NPC] = x[c * NPC:(c + 1) * NPC].T
        m = dict(common)
        m["xT"] = _bf16(xT)
        m["gidx"] = pp["idx_all"][c]
        m["dstrel"] = _bf16(pp["dstrel_all"][c])
        m["rcnt_nm"] = pp["rcnt_nm"][c]
        m["rcnt_fm"] = np.broadcast_to(pp["rcnt_row"][c][None, :], (64, NPAD)).copy()
        in_maps.append(m)
    return in_maps


def kernel(**inputs):
    edge_index = np.asarray(inputs["edge_index"])
    pp = _preprocess(edge_index)
    key = (pp["npieces"], pp["nch_call"].tobytes())
    if key not in _PROG_CACHE:
        _PROG_CACHE[key] = _build_program(pp)
    nc = _PROG_CACHE[key]
    in_maps = _in_maps(pp, inputs)
    from concourse.bass_utils import run_bass_kernel_spmd
    res = run_bass_kernel_spmd(nc, in_maps, core_ids=list(range(NCORES)))
    return np.concatenate([res.results[c]["out"][:NPC] for c in range(NCORES)], axis=0)
